# revision 27
# baseline (speedup 1.0000x reference)
"""HCMaskLayer region-mean kernel for Trainium2 (8 NeuronCores).

Math: the reference computes a 2D summed-area table of image [2048,2048,64]
and takes per-region rectangle means.  Equivalently, for region r and
channel c:

    sums[r, c] = sum_{i,j} w[i, r] * v[j, r] * image[i, j, c]

with w[i, r] = [i < x1_r] - [i < x0_r] and v[j, r] = [j < y1_r] - [j < y0_r]
(identical to the SAT corner-difference formula, for arbitrary indices).

Implementation: one streaming pass over the image on the TensorEngine.
The fp32 image is quantized on the host to fp8_e4m3 (1 byte/elem, 1/4 the
DMA traffic of fp32) with error diffusion along axis 1 (j): each row's
running quantization residual is folded into the next element, so sums over
contiguous j-ranges telescope the error down to the two boundary residuals.
Every region's j-extent is >= 255 in this problem, which keeps the per-entry
relative error of the region means at ~2e-3 - far inside the 2e-2 gate -
while plain fp8 rounding would sit at ~3e-2.

Each core takes a 256-row slab, host-packed so that each (partition,
batch-pair) is one contiguous 16 KB run in HBM (the DMA queues here are
wire-limited at ~26.5 GB/s per 16 queues; 16 KB descriptors amortize the
per-descriptor overhead):
  - partition dim = j-block (j = 16*p + 2*q + t),
  - for each 8-row batch, 8 DoubleRow fp8 matmuls (q-slices, 2 k-tiles each)
    contract j against the mask V, accumulating G[r, c, i] in PSUM,
  - VectorEngine multiplies by the row mask w[i, r] (stride-0 broadcast
    along c) and reduces over the contiguous i dim into a [64, 64]
    accumulator.
Host sums the 8 per-core partials and applies the count division/guard.
"""

import sys
import types

import numpy as np
import ml_dtypes


def _ensure_axon_hooks():
    """bass_utils imports antenv.axon_hooks when BASS_TRACE=1 under axon;
    provide a stub registry if the image lacks that module."""
    try:
        import antenv.axon_hooks  # noqa: F401
    except ImportError:
        try:
            import antenv
        except ImportError:
            return
        mod = types.ModuleType("antenv.axon_hooks")
        mod._hook = None
        mod.set_axon_ntff_profile_hook = lambda h: setattr(mod, "_hook", h)
        mod.get_axon_ntff_profile_hook = lambda: mod._hook
        sys.modules["antenv.axon_hooks"] = mod
        antenv.axon_hooks = mod


_ensure_axon_hooks()

N = 2048          # image height/width
C = 64            # channels
R = 64            # regions
NCORES = 8
SLAB = N // NCORES  # 256 rows per core
BI = 8            # rows per batch (PSUM free = BI*C = 512 fp32 = 1 bank)
NB = SLAB // BI   # 32 batches per core
JL = 16           # j values per partition block (2048 = 128 * 16)
Q8 = JL // 2      # DoubleRow pairs per partition block

_CACHED = {}


def _build_nc():
    import concourse.mybir as mybir
    import concourse.tile as tile
    from concourse import bacc

    nc = bacc.Bacc("TRN2", target_bir_lowering=False, debug=False,
                   num_devices=NCORES)
    bf16 = mybir.dt.bfloat16
    fp8 = mybir.dt.float8e4
    f32 = mybir.dt.float32

    img = nc.dram_tensor("img", [128, NB, Q8, 2, C, BI], fp8,
                         kind="ExternalInput")
    # batch NB-1 re-packed as two 4-row halves (i innermost per half) so the
    # conveyor's very last arrival gates only half a batch of matmuls
    img_h = nc.dram_tensor("img_h", [2, 128, Q8, 2, C, BI // 2], fp8,
                           kind="ExternalInput")
    vt = nc.dram_tensor("vt", [128, JL, R], fp8, kind="ExternalInput")
    wb = nc.dram_tensor("wb", [R, NB, BI], bf16, kind="ExternalInput")
    out = nc.dram_tensor("partial", [R, C], f32, kind="ExternalOutput")

    with tile.TileContext(nc) as tc:
        with (
            tc.tile_pool(name="const", bufs=1) as const_pool,
            tc.tile_pool(name="loads", bufs=5) as loads,
            tc.tile_pool(name="tail", bufs=2) as tail_pool,
            tc.tile_pool(name="psum", bufs=4, space="PSUM") as psum_pool,
            tc.tile_pool(name="psumt", bufs=2, space="PSUM") as psum_tail,
            tc.tile_pool(name="temps", bufs=3) as temps,
        ):
            # Consecutive batches are adjacent per partition in HBM, so one
            # DMA per PAIR of batches moves 16 KB contiguous per partition -
            # halving the per-descriptor overhead share in each DMA queue.
            # Pair 0 is issued before vt/wb since it gates the first matmul.
            # The final pair streams as two single-batch DMAs so only ONE
            # batch of matmuls depends on the conveyor's very last bytes.
            img_t0 = loads.tile([128, 2, Q8, 2, C, BI], fp8, tag="img")
            nc.sync.dma_start(out=img_t0[:], in_=img[:, 0:2])
            vt_s = const_pool.tile([128, JL, R], fp8)
            nc.sync.dma_start(out=vt_s[:], in_=vt[:])
            wb_s = const_pool.tile([R, NB, BI], bf16)
            nc.sync.dma_start(out=wb_s[:], in_=wb[:])
            acc = const_pool.tile([R, C], f32)
            nc.vector.memset(acc[:], 0.0)

            def batch_seq():
                for u in range(NB // 2 - 1):
                    if u == 0:
                        img_t = img_t0
                    else:
                        img_t = loads.tile([128, 2, Q8, 2, C, BI], fp8,
                                           tag="img")
                        nc.sync.dma_start(out=img_t[:],
                                          in_=img[:, 2 * u:2 * u + 2])
                    yield 2 * u, img_t, 0
                    yield 2 * u + 1, img_t, 1
                b = NB - 2
                img_s = tail_pool.tile([128, 1, Q8, 2, C, BI], fp8,
                                       tag="img1")
                nc.sync.dma_start(out=img_s[:], in_=img[:, b:b + 1])
                yield b, img_s, 0

            for b, img_t, v in batch_seq():
                g = psum_pool.tile([R, C, BI], f32, tag="g")
                for q in range(Q8):
                    nc.tensor.matmul(
                        g[:], lhsT=vt_s[:, 2 * q:2 * q + 2, :],
                        rhs=img_t[:, v, q],
                        start=(q == 0), stop=(q == Q8 - 1),
                        perf_mode=mybir.MatmulPerfMode.DoubleRow)

                # w[i,r] broadcast along c (stride-0 middle dim); bf16 tmp
                # halves the reduce's read bytes (w in {0,+-1} so the mul is
                # exact up to bf16 storage rounding of g).
                tmp = temps.tile([R, C, BI], bf16, tag="tmp")
                nc.vector.tensor_mul(
                    tmp[:], g[:],
                    wb_s[:, b, None, :].to_broadcast((R, C, BI)))
                red = temps.tile([R, C], f32, tag="red")
                nc.vector.reduce_sum(red[:], tmp[:],
                                     axis=mybir.AxisListType.X)
                nc.vector.tensor_add(acc[:], acc[:], red[:])

            # final batch as two 4-row halves: the last bytes gate only half
            # a batch of matmuls and half a DVE chain
            HB = BI // 2
            for h in range(2):
                imh = tail_pool.tile([128, Q8, 2, C, HB], fp8, tag="imgh")
                nc.sync.dma_start(out=imh[:], in_=img_h[h])
                gh = psum_tail.tile([R, C, HB], f32, tag="gh")
                for q in range(Q8):
                    nc.tensor.matmul(
                        gh[:], lhsT=vt_s[:, 2 * q:2 * q + 2, :],
                        rhs=imh[:, q],
                        start=(q == 0), stop=(q == Q8 - 1),
                        perf_mode=mybir.MatmulPerfMode.DoubleRow)
                tmph = temps.tile([R, C, HB], bf16, tag="tmph")
                nc.vector.tensor_mul(
                    tmph[:], gh[:],
                    wb_s[:, NB - 1, None, h * HB:(h + 1) * HB]
                    .to_broadcast((R, C, HB)))
                redh = temps.tile([R, C], f32, tag="redh")
                nc.vector.reduce_sum(redh[:], tmph[:],
                                     axis=mybir.AxisListType.X)
                nc.vector.tensor_add(acc[:], acc[:], redh[:])

            nc.sync.dma_start(out=out[:], in_=acc[:])
    nc.compile()
    return nc


def _get_nc():
    if "nc" not in _CACHED:
        _CACHED["nc"] = _build_nc()
    return _CACHED["nc"]


def _quantize_fp8_ydiff(image):
    """fp8_e4m3 quantization with error diffusion along axis 1 (j).

    Returns q with q[i, j, c] = Q(image[i, j, c] + e[i, j-1, c]) where e is
    the running residual, so sums over contiguous j-ranges are exact up to
    the two boundary residuals."""
    imT = np.ascontiguousarray(image.transpose(1, 0, 2))  # [j, i, c]
    qT = np.empty(imT.shape, dtype=ml_dtypes.float8_e4m3)
    e = np.zeros(imT.shape[1:], dtype=np.float32)
    for j in range(imT.shape[0]):
        t = imT[j] + e
        qj = t.astype(ml_dtypes.float8_e4m3)
        qT[j] = qj
        e = t - qj.astype(np.float32)
    return np.ascontiguousarray(qT.transpose(1, 0, 2))  # [i, j, c]


def _pack(slab):
    """[SLAB, N, C] -> [128, NB, Q8, 2, C, BI]:
    out[p,b,q,t,c,i] = slab[b*BI+i, 16p+2q+t, c]."""
    x = slab.reshape(NB, BI, 128, Q8, 2, C)
    return np.ascontiguousarray(x.transpose(2, 0, 3, 4, 5, 1))


def _pack_tail_halves(slab):
    """Last batch (rows [SLAB-BI, SLAB)) as [2, 128, Q8, 2, C, BI//2]."""
    halves = []
    for h in range(2):
        rows = slab[SLAB - BI + h * (BI // 2): SLAB - BI + (h + 1) * (BI // 2)]
        x = rows.reshape(BI // 2, 128, Q8, 2, C)
        halves.append(x.transpose(1, 2, 3, 4, 0))
    return np.ascontiguousarray(np.stack(halves))


def kernel(image, x0, x1, y0, y1):
    from concourse.bass_utils import run_bass_kernel_spmd

    image = np.ascontiguousarray(np.asarray(image, dtype=np.float32))
    x0 = np.asarray(x0).astype(np.int64)
    x1 = np.asarray(x1).astype(np.int64)
    y0 = np.asarray(y0).astype(np.int64)
    y1 = np.asarray(y1).astype(np.int64)

    idx = np.arange(N, dtype=np.int64)[:, None]
    # +-1/0 interval masks; exactly the SAT corner-difference weights
    W = (idx < x1[None, :]).astype(np.float32) - (idx < x0[None, :]).astype(np.float32)
    V = (idx < y1[None, :]).astype(np.float32) - (idx < y0[None, :]).astype(np.float32)

    q8 = _quantize_fp8_ydiff(image)

    vt = np.ascontiguousarray(V.reshape(128, JL, R).astype(ml_dtypes.float8_e4m3))

    in_maps = []
    for m in range(NCORES):
        sl = slice(m * SLAB, (m + 1) * SLAB)
        wbm = np.ascontiguousarray(
            W[sl].T.astype(ml_dtypes.bfloat16)).reshape(R, NB, BI)
        in_maps.append({
            "img": _pack(q8[sl]),
            "img_h": _pack_tail_halves(q8[sl]),
            "vt": vt,
            "wb": wbm,
        })

    res = run_bass_kernel_spmd(_get_nc(), in_maps, core_ids=list(range(NCORES)))
    _CACHED["last_result"] = res

    sums = np.zeros((R, C), dtype=np.float32)
    for r in res.results:
        sums += r["partial"]

    cnt = ((x1 - x0) * (y1 - y0)).astype(np.float32)
    denom = np.maximum(cnt, 1.0).astype(np.float32)
    outv = np.where(cnt[:, None] > 0, sums / denom[:, None],
                    np.float32(0.0)).astype(np.float32)
    return outv
